# revision 38
# baseline (speedup 1.0000x reference)
"""Equivariant MPNN layer as a Bass/Tile kernel for TRN2 (v2).

Strategy:
  - Edges sorted by destination grid cell (j), sharded across 8 cores by
    contiguous 128-seg blocks (4096 segs / core). Per core, the 32 blocks
    are permuted by descending edge count so every core shares one static
    tile schedule (per-slot tile count = cross-core max at that rank);
    host un-permutes the output columns.
  - All per-edge operands streamed from host in fp16 (1 cyc/row matmuls):
      zp [12, e_pad]: rows 0:9 = R[a,b]*gp[b], rows 9:12 = R@np  (mm1 is a
        single K=12 matmul with W1big = [repeat(We1,3); -We1])
      dt [128, ntiles*H]: D[i] = node_emb[i] @ Wm1_top + (bm1 + be2@Wm1_bot),
        gathered per edge on host, tiled so each chunk DMA is contiguous.
  - pm[e,h'] = h1 @ Wf + D[i] with Wf = We2 @ Wm1_bot; the D-add is folded
    into the PE accumulation group via an identity-weight matmul (no DVE).
  - msg' = silu(pm); segment sums via one-hot matmul with msg' STATIONARY
    (lhsT) and the one-hot as moving operand -> psum lands pre-transposed
    [h, seg]; one DVE copy per block into ST_all.
  - Post (per 512 segs): pmm = Wm2^T ST + bm2 (x) cnt; mean = pmm * INVB
    (INVB = 1/max(cnt,1) broadcast, host const); update MLP in fp32.
Outputs are [128, SEGS_PER_CORE] (slot order, transposed); host reassembles.
"""

import math
from contextlib import ExitStack

import numpy as np

import concourse.bass as bass
import concourse.tile as tile
from concourse import bacc, mybir

F32 = mybir.dt.float32
F16 = mybir.dt.float16
AOT = mybir.AluOpType

H = 128
P = 128
BLK = 128          # segments per psum block
CHUNK_TILES = 16   # tiles per DMA chunk (2048 edges)


class Cfg:
    def __init__(self, N, G, E, B, ncores, slot_tiles, reps=1, loop_k=1):
        self.N, self.G, self.E, self.B = N, G, E, B
        self.ncores = ncores
        self.loop_k = loop_k
        self.reps = reps
        self.segs_core = G // ncores
        self.nslots = self.segs_core // BLK
        self.slot_tiles = list(slot_tiles)      # tiles per slot (static)
        assert len(self.slot_tiles) == self.nslots
        self.ntiles = sum(self.slot_tiles)
        assert self.ntiles % CHUNK_TILES == 0
        self.e_pad = self.ntiles * P
        self.nchunks = self.ntiles // CHUNK_TILES
        # per-tile slot id + first/last flags
        self.tile_slot = []
        self.tile_first = []
        self.tile_last = []
        for s, nt in enumerate(self.slot_tiles):
            for i in range(nt):
                self.tile_slot.append(s)
                self.tile_first.append(i == 0)
                self.tile_last.append(i == nt - 1)

    def key(self):
        return (self.ntiles, tuple(self.slot_tiles), self.loop_k, self.reps)


def build_program(cfg: Cfg):
    nc = bacc.Bacc("TRN2", target_bir_lowering=False, debug=False,
                   num_devices=cfg.ncores)

    def din(name, shape, dt=F32):
        return nc.dram_tensor(name, shape, dt, kind="ExternalInput").ap()

    W1q = din("W1q", [P, H], F16)       # We1 rows replicated at partitions 32j+a
    Wf = din("Wf", [H, H], F16)
    Ie = din("Ie", [P, P], F16)
    IOTA = din("IOTA", [P, P], F16)
    be1c = din("be1c", [H, 1])
    Wm2 = din("Wm2", [H, H])
    bm2r = din("bm2r", [1, H])
    Wu1 = din("Wu1", [H, H])
    bu1c = din("bu1c", [H, 1])
    Wu2 = din("Wu2", [H, H])
    bu2c = din("bu2c", [H, 1])
    if cfg.loop_k > 1:
        din("ktag", [1, cfg.loop_k])             # shape tag to defeat HLO cache
    zp = din("zp", [P, cfg.e_pad // 4], F16)   # local_rel, 4 groups x partitions
    dt = din("dt", [P, cfg.ntiles * H], F16)
    segf = din("segf", [P, cfg.ntiles], F32)
    cntin = din("cntin", [1, cfg.segs_core], F32)
    invb = din("invb", [P, cfg.segs_core], F32)

    outT = nc.dram_tensor("outT", [H, cfg.segs_core], F32,
                          kind="ExternalOutput").ap()
    dbg = {}
    if getattr(cfg, "debug_dump", False):
        for nm, shp in [("d_pre1", [H, 1024]), ("d_h1", [H, 1024]),
                        ("d_pm", [P, 512]), ("d_msgp", [P, 512]),
                        ("d_at", [P, P]), ("d_ST", [H, cfg.segs_core])]:
            dbg[nm] = nc.dram_tensor(nm, shp, F32,
                                     kind="ExternalOutput").ap()

    with tile.TileContext(nc) as tc, ExitStack() as ctx:
        ep = ctx.enter_context

        consts = ep(tc.tile_pool(name="consts", bufs=1))
        zpool = ep(tc.tile_pool(name="zpool", bufs=4))
        gpool = ep(tc.tile_pool(name="gpool", bufs=4))
        hpool = ep(tc.tile_pool(name="hpool", bufs=4))
        mpool = ep(tc.tile_pool(name="mpool", bufs=6))
        apool = ep(tc.tile_pool(name="apool", bufs=12))
        spool = ep(tc.tile_pool(name="spool", bufs=1))
        postp = ep(tc.tile_pool(name="postp", bufs=4))
        # pre1 and pm share one 3-slot rotation (tag "ps", 6 banks): their
        # lifetimes interleave (pre1[k] dies at silu1[k], pm[k] at silu2[k])
        # so 3 slots give both double-buffering within 6 banks, leaving 2
        # banks for the segment accumulators.
        pspool = ep(tc.tile_pool(name="pspool", bufs=3, space="PSUM"))
        psS = ep(tc.tile_pool(name="psS", bufs=2, space="PSUM"))       # seg acc

        def csb(ap_in, shape, dt_=F32, tag=None):
            t = consts.tile(shape, dt_, tag=tag or ap_in.tensor.name)
            nc.sync.dma_start(t[:], ap_in)
            return t

        W1q_sb = csb(W1q, [P, H], F16)
        Wf_sb = csb(Wf, [H, H], F16)
        Ie_sb = csb(Ie, [P, P], F16)
        IOTA_sb = csb(IOTA, [P, P], F16)
        be1_sb = csb(be1c, [H, 1])
        Wm2_sb = csb(Wm2, [H, H])
        bm2_sb = csb(bm2r, [1, H])
        Wu1_sb = csb(Wu1, [H, H])
        bu1_sb = csb(bu1c, [H, 1])
        Wu2_sb = csb(Wu2, [H, H])
        bu2_sb = csb(bu2c, [H, 1])
        segf_sb = consts.tile([P, cfg.ntiles], F32, tag="segf")
        nc.sync.dma_start(segf_sb[:], segf)
        cnt_sb = consts.tile([1, cfg.segs_core], F32, tag="cnt")
        nc.sync.dma_start(cnt_sb[:], cntin)
        invb_sb = consts.tile([P, cfg.segs_core], F32, tag="invb")
        nc.sync.dma_start(invb_sb[:], invb)

        def post_chunk(ST_all, u):
            """mean -> update MLP for segs [u*512, (u+1)*512)."""
            s0 = u * 512
            pmm = pspool.tile([P, 512], F32, tag="ps")
            nc.tensor.matmul(out=pmm[:], lhsT=Wm2_sb[:],
                             rhs=ST_all[:, s0:s0 + 512],
                             start=True, stop=False)
            nc.tensor.matmul(out=pmm[:], lhsT=bm2_sb[:],
                             rhs=cnt_sb[0:1, s0:s0 + 512],
                             start=False, stop=True)
            mean = postp.tile([H, 512], F32, tag="mean")
            nc.vector.tensor_tensor(out=mean[:], in0=pmm[:],
                                    in1=invb_sb[:, s0:s0 + 512],
                                    op=AOT.mult)
            pu = pspool.tile([P, 512], F32, tag="ps")
            nc.tensor.matmul(out=pu[:], lhsT=Wu1_sb[:], rhs=mean[:],
                             start=True, stop=True)
            hu = postp.tile([H, 512], F32, tag="hu")
            nc.scalar.activation(out=hu[:], in_=pu[:],
                                 func=mybir.ActivationFunctionType.Silu,
                                 bias=bu1_sb[:])
            po = pspool.tile([P, 512], F32, tag="ps")
            nc.tensor.matmul(out=po[:], lhsT=Wu2_sb[:], rhs=hu[:],
                             start=True, stop=True)
            ot = postp.tile([H, 512], F32, tag="ot")
            nc.vector.tensor_scalar(out=ot[:], in0=po[:],
                                    scalar1=bu2_sb[:, :1], scalar2=None,
                                    op0=AOT.add)
            nc.sync.dma_start(outT[:, s0:s0 + 512], ot[:])

        loop_cm = tc.For_i(0, cfg.loop_k, 1) if cfg.loop_k > 1 else None
        if loop_cm is not None:
            ctx.enter_context(loop_cm)
        for rep in range(cfg.reps):
            ST_all = spool.tile([H, cfg.segs_core], F32, tag="ST")
            state = {"ps_blk": None}
            nbatch = cfg.nchunks * 2          # 1024-edge batches
            hmap = {}
            dmap = {}

            def stage_a(k):
                """DMA (per chunk) + mm1 + silu1 for batch k."""
                c, g = divmod(k, 2)
                if g == 0:
                    zcol = c * CHUNK_TILES * P // 4
                    zpc = zpool.tile([99, CHUNK_TILES * P // 4], F16,
                                     tag="zpc")
                    nc.sync.dma_start(
                        zpc[:], zp[0:99, zcol:zcol + CHUNK_TILES * P // 4])
                    dtc = gpool.tile([P, CHUNK_TILES * H], F16, tag="dtc")
                    nc.sync.dma_start(dtc[:],
                                      dt[:, c * CHUNK_TILES * H:
                                         (c + 1) * CHUNK_TILES * H])
                    dmap[c] = (zpc, dtc)
                zpc, dtc = dmap[c]
                pre1 = pspool.tile([H, 1024], F32, tag="ps")
                # two row-tiled K=3 matmuls run concurrently on the PE
                for jj in range(2):
                    r0 = 64 * g + 32 * jj
                    nc.tensor.matmul(out=pre1[:, jj * 512:(jj + 1) * 512],
                                     lhsT=W1q_sb[r0:r0 + 3, :],
                                     rhs=zpc[r0:r0 + 3, :],
                                     start=True, stop=True,
                                     tile_position=(r0, 0))
                h1 = hpool.tile([H, 1024], F16, tag="h1")
                nc.scalar.activation(out=h1[:], in_=pre1[:],
                                     func=mybir.ActivationFunctionType.Silu,
                                     bias=be1_sb[:])
                hmap[k] = h1

            def stage_b(k):
                """mm2 + silu2 + segment reduction for batch k."""
                c, g = divmod(k, 2)
                _, dtc = dmap[c]
                h1 = hmap.pop(k)
                # one pm tile spanning 2 banks; per bank: D matmul first
                # with start=True (start clears has_written bank-wide),
                # then the Wf matmuls accumulate per-quarter.
                pm = pspool.tile([P, 1024], F32, tag="ps")
                for half in range(2):
                    hofs = half * 512
                    dts = dtc[:, (g * 8 + half * 4) * H:
                              (g * 8 + half * 4 + 4) * H]
                    nc.tensor.matmul(out=pm[:, hofs:hofs + 512],
                                     lhsT=Ie_sb[:], rhs=dts,
                                     start=True, stop=False,
                                     skip_group_check=True)
                    for t4 in range(4):
                        co = hofs + t4 * 128
                        nc.tensor.matmul(
                            out=pm[:, co:co + 128],
                            lhsT=h1[:, co:co + 128],
                            rhs=Wf_sb[:], start=False, stop=True,
                            skip_group_check=True)
                msgp = mpool.tile([P, 1024], F16, tag="msgp")
                nc.scalar.activation(
                    out=msgp[:], in_=pm[:],
                    func=mybir.ActivationFunctionType.Silu)
                t0 = c * CHUNK_TILES + g * 8
                for t8 in range(8):
                    t = t0 + t8
                    s = cfg.tile_slot[t]
                    at = apool.tile([P, P], F16, tag="at")
                    nc.vector.tensor_scalar(
                        out=at[:], in0=IOTA_sb[:],
                        scalar1=segf_sb[:, t:t + 1], scalar2=None,
                        op0=AOT.is_equal)
                    if cfg.tile_first[t]:
                        ps_new = psS.tile([H, P], F32, tag="psS")
                        state["ps_blk"] = ps_new
                    nc.tensor.matmul(
                        out=state["ps_blk"][:],
                        lhsT=msgp[:, t8 * 128:t8 * 128 + 128],
                        rhs=at[:],
                        start=cfg.tile_first[t],
                        stop=cfg.tile_last[t])
                    if cfg.tile_last[t]:
                        nc.vector.tensor_copy(
                            out=ST_all[:, s * BLK:(s + 1) * BLK],
                            in_=state["ps_blk"][:])
                        # post for a 512-seg range as soon as its
                        # 4 slots are all reduced
                        if (s + 1) % 4 == 0:
                            post_chunk(ST_all, s // 4)

            # software pipeline: stage_a one batch ahead of stage_b
            for k in range(nbatch + 1):
                if k < nbatch:
                    stage_a(k)
                if k >= 1:
                    stage_b(k - 1)

            if dbg and rep == 0:
                nc.sync.dma_start(dbg["d_ST"], ST_all[:])

    nc.compile()
    return nc


# ======================= host preprocessing =======================

def silu_np(x):
    return x / (1.0 + np.exp(-x))


def host_prep(inputs, ncores, use_bf16=False, t_override=None):
    """Returns (cfg, list of per-core in_maps, const row for node outputs)."""
    nemb = np.asarray(inputs["node_embedding"], np.float32)
    npos = np.asarray(inputs["node_pos"], np.float32)
    gpos = np.asarray(inputs["grid_pos"], np.float32)
    eidx = np.asarray(inputs["edge_index"], np.int64)
    frames = np.asarray(inputs["equi_frames"], np.float32)
    batch = np.asarray(inputs["batch"], np.int64)
    We1 = np.asarray(inputs["We1"], np.float32); be1 = np.asarray(inputs["be1"], np.float32)
    We2 = np.asarray(inputs["We2"], np.float32); be2 = np.asarray(inputs["be2"], np.float32)
    Wm1 = np.asarray(inputs["Wm1"], np.float32); bm1 = np.asarray(inputs["bm1"], np.float32)
    Wm2 = np.asarray(inputs["Wm2"], np.float32); bm2 = np.asarray(inputs["bm2"], np.float32)
    Wu1 = np.asarray(inputs["Wu1"], np.float32); bu1 = np.asarray(inputs["bu1"], np.float32)
    Wu2 = np.asarray(inputs["Wu2"], np.float32); bu2 = np.asarray(inputs["bu2"], np.float32)

    N, Hh = nemb.shape
    G = gpos.shape[0]
    E = eidx.shape[1]
    B = frames.shape[0]
    assert Hh == H

    i_all = eidx[0]
    jg_all = eidx[1] - N
    order = np.argsort(jg_all, kind="stable")
    jg_s = jg_all[order]
    i_s = i_all[order]

    segs_core = G // ncores
    nslots = segs_core // BLK
    nblk_g = G // BLK
    gb = jg_s // BLK                                  # global block per edge
    counts_blk = np.bincount(gb, minlength=nblk_g)
    per_core_cnt = counts_blk.reshape(ncores, nslots)

    # per-core block permutation: descending count; shared static schedule
    perms = [np.argsort(per_core_cnt[c], kind="stable")[::-1]
             for c in range(ncores)]
    sorted_cnt = np.stack([per_core_cnt[c][perms[c]] for c in range(ncores)])
    slot_tiles = np.ceil(sorted_cnt.max(axis=0) / P).astype(int)
    slot_tiles = np.maximum(slot_tiles, 1)
    ntiles = int(slot_tiles.sum())
    pad = (-ntiles) % CHUNK_TILES
    slot_tiles[-1] += pad
    cfg = Cfg(N, G, E, B, ncores, slot_tiles.tolist())
    slot_off = np.zeros(nslots + 1, np.int64)
    slot_off[1:] = np.cumsum(slot_tiles)

    # per-edge host data (pure data movement + O(N)/O(B)/O(E) elementwise math)
    R_flat = frames.reshape(B, 9)
    b_e = batch[i_s]
    gp_e = gpos[jg_s]                                  # [E, 3]
    zr = R_flat[b_e].reshape(E, 3, 3)                  # [E, 3, 3]
    rp_node = np.einsum("nab,nb->na", frames[batch], npos).astype(np.float32)
    lr_e = (zr * gp_e[:, None, :]).sum(-1) - rp_node[i_s]   # local_rel [E, 3]

    bmix = bm1 + be2 @ Wm1[H:]
    Dtab = (nemb @ Wm1[:H] + bmix[None, :]).astype(np.float32)   # [N, H]
    D_e = Dtab[i_s]                                    # [E, H]

    ecount = np.bincount(jg_all, minlength=G).astype(np.float32)

    W1q = np.zeros((P, H), np.float16)
    for J in range(4):
        W1q[32 * J:32 * J + 3] = We1.astype(np.float16)
    shared = {
        "W1q": W1q,
        "Wf": np.ascontiguousarray(We2 @ Wm1[H:]).astype(np.float16),
        "Ie": np.eye(P, dtype=np.float16),
        "IOTA": np.ascontiguousarray(
            np.tile(np.arange(P, dtype=np.float16)[None, :], (P, 1))),
        "be1c": np.ascontiguousarray(be1[:, None]),
        "Wm2": np.ascontiguousarray(Wm2),
        "bm2r": np.ascontiguousarray(bm2[None, :]),
        "Wu1": np.ascontiguousarray(Wu1),
        "bu1c": np.ascontiguousarray(bu1[:, None]),
        "Wu2": np.ascontiguousarray(Wu2),
        "bu2c": np.ascontiguousarray(bu2[:, None]),
    }

    core_of_edge = gb // nslots
    in_maps = []
    for c in range(ncores):
        sel = core_of_edge == c
        jg_c = jg_s[sel]
        b_local = (gb[sel] % nslots)                   # original block id
        # slot of each edge + position within slot
        inv_perm = np.empty(nslots, np.int64)
        inv_perm[perms[c]] = np.arange(nslots)
        slot_e = inv_perm[b_local]
        # rank within block: edges sorted by jg so within-block order stable
        cnts = per_core_cnt[c][perms[c]]
        # compute start offset of each edge within its block
        blk_start = np.zeros(nblk_g + 1, np.int64)
        blk_start[1:] = np.cumsum(counts_blk)
        rank = np.nonzero(sel)[0] - blk_start[gb[sel]]
        slot_pos = slot_off[slot_e] * P + rank

        e_pad = cfg.e_pad
        lr_c = np.zeros((e_pad, 3), np.float32)
        lr_c[slot_pos] = lr_e[sel]
        dt_c = np.zeros((e_pad, H), np.float32)
        dt_c[slot_pos] = D_e[sel]
        segf_f = np.full(e_pad, -1.0, np.float32)
        segf_f[slot_pos] = (jg_c % BLK).astype(np.float32)

        # device layouts: zp_t[32J + a, c*512 + col] = lr_a of edge
        # c*2048 + J*512 + col (partition-packed quarters of each chunk)
        v = lr_c.reshape(cfg.nchunks, 4, 512, 3)
        zp_t = np.zeros((P, e_pad // 4), np.float16)
        for J in range(4):
            for a in range(3):
                zp_t[32 * J + a] = v[:, J, :, a].reshape(-1)
        dt_t = np.ascontiguousarray(
            dt_c.reshape(cfg.ntiles, P, H).transpose(1, 0, 2)
            .reshape(P, cfg.ntiles * H)).astype(np.float16)
        segf_t = np.ascontiguousarray(
            segf_f.reshape(cfg.ntiles, P).T)                     # [128, ntiles]

        cnt_core = ecount[c * segs_core:(c + 1) * segs_core]
        cnt_slot = cnt_core.reshape(nslots, BLK)[perms[c]].reshape(-1)
        inv_slot = 1.0 / np.maximum(cnt_slot, 1.0)

        m = dict(shared)
        m["zp"] = zp_t
        m["dt"] = dt_t
        m["segf"] = segf_t
        m["cntin"] = np.ascontiguousarray(cnt_slot[None, :].astype(np.float32))
        m["invb"] = np.ascontiguousarray(
            np.tile(inv_slot[None, :], (P, 1)).astype(np.float32))
        in_maps.append(m)

    const_row = silu_np(bu1) @ Wu2 + bu2
    cfg.perms = perms
    return cfg, in_maps, const_row


def assemble_output(cfg, results, const_row, N, G):
    out = np.empty((N + G, H), np.float32)
    out[:N] = const_row[None, :]
    nslots = cfg.nslots
    for c in range(cfg.ncores):
        res = results[c]["outT"].T                    # [segs_core, H] slot order
        dest = out[N + c * cfg.segs_core: N + (c + 1) * cfg.segs_core]
        dest.reshape(nslots, BLK, H)[cfg.perms[c]] = res.reshape(nslots, BLK, H)
    return out


# ======================= top-level kernel entry =======================

_PROGRAM_CACHE = {}

NCORES = 8
USE_BF16 = False


def kernel(**inputs):
    """Full-input entry point: shards edges by destination grid cell across
    8 NeuronCores, runs the Bass/Tile program, reassembles the full output."""
    from concourse.bass_utils import run_bass_kernel_spmd

    cfg, in_maps, const_row = host_prep(inputs, NCORES, use_bf16=USE_BF16)
    key = cfg.key()
    if key not in _PROGRAM_CACHE:
        _PROGRAM_CACHE[key] = build_program(cfg)
    nc = _PROGRAM_CACHE[key]
    res = run_bass_kernel_spmd(nc, in_maps, core_ids=list(range(NCORES)))
    N = inputs["node_pos"].shape[0]
    G = inputs["grid_pos"].shape[0]
    return assemble_output(cfg, res.results, const_row, N, G)


# revision 42
# speedup vs baseline: 1.2335x; 1.2335x over previous
"""Equivariant MPNN layer as a Bass/Tile kernel for TRN2 (v2).

Strategy:
  - Edges sorted by destination grid cell (j), sharded across 8 cores by
    contiguous 128-seg blocks (4096 segs / core). Per core, the 32 blocks
    are permuted by descending edge count so every core shares one static
    tile schedule (per-slot tile count = cross-core max at that rank);
    host un-permutes the output columns.
  - All per-edge operands streamed from host in fp16 (1 cyc/row matmuls):
      zp [12, e_pad]: rows 0:9 = R[a,b]*gp[b], rows 9:12 = R@np  (mm1 is a
        single K=12 matmul with W1big = [repeat(We1,3); -We1])
      dt [128, ntiles*H]: D[i] = node_emb[i] @ Wm1_top + (bm1 + be2@Wm1_bot),
        gathered per edge on host, tiled so each chunk DMA is contiguous.
  - pm[e,h'] = h1 @ Wf + D[i] with Wf = We2 @ Wm1_bot; the D-add is folded
    into the PE accumulation group via an identity-weight matmul (no DVE).
  - msg' = silu(pm); segment sums via one-hot matmul with msg' STATIONARY
    (lhsT) and the one-hot as moving operand -> psum lands pre-transposed
    [h, seg]; one DVE copy per block into ST_all.
  - Post (per 512 segs): pmm = Wm2^T ST + bm2 (x) cnt; mean = pmm * INVB
    (INVB = 1/max(cnt,1) broadcast, host const); update MLP in fp32.
Outputs are [128, SEGS_PER_CORE] (slot order, transposed); host reassembles.
"""

import math
from contextlib import ExitStack

import numpy as np

import concourse.bass as bass
import concourse.tile as tile
from concourse import bacc, mybir

F32 = mybir.dt.float32
F16 = mybir.dt.float16
AOT = mybir.AluOpType

H = 128
P = 128
BLK = 128          # segments per psum block
CHUNK_TILES = 16   # tiles per DMA chunk (2048 edges)


class Cfg:
    def __init__(self, N, G, E, B, ncores, slot_tiles, reps=1, loop_k=1):
        self.N, self.G, self.E, self.B = N, G, E, B
        self.ncores = ncores
        self.loop_k = loop_k
        self.reps = reps
        self.segs_core = G // ncores
        self.nslots = self.segs_core // BLK
        self.slot_tiles = list(slot_tiles)      # tiles per slot (static)
        assert len(self.slot_tiles) == self.nslots
        self.ntiles = sum(self.slot_tiles)
        assert self.ntiles % CHUNK_TILES == 0
        self.e_pad = self.ntiles * P
        self.nchunks = self.ntiles // CHUNK_TILES
        # per-tile slot id + first/last flags
        self.tile_slot = []
        self.tile_first = []
        self.tile_last = []
        for s, nt in enumerate(self.slot_tiles):
            for i in range(nt):
                self.tile_slot.append(s)
                self.tile_first.append(i == 0)
                self.tile_last.append(i == nt - 1)

    def key(self):
        return (self.ntiles, tuple(self.slot_tiles), self.loop_k, self.reps)


def build_program(cfg: Cfg):
    nc = bacc.Bacc("TRN2", target_bir_lowering=False, debug=False,
                   num_devices=cfg.ncores)

    def din(name, shape, dt=F32):
        return nc.dram_tensor(name, shape, dt, kind="ExternalInput").ap()

    W1q = din("W1q", [P, H], F16)       # We1 rows replicated at partitions 32j+a
    Wf = din("Wf", [H, H], F16)
    Ie = din("Ie", [P, P], F16)
    IOTA = din("IOTA", [P, P], F16)
    be1c = din("be1c", [H, 1])
    Wm2 = din("Wm2", [H, H])
    bm2r = din("bm2r", [1, H])
    Wu1 = din("Wu1", [H, H])
    bu1c = din("bu1c", [H, 1])
    Wu2 = din("Wu2", [H, H])
    bu2c = din("bu2c", [H, 1])
    if cfg.loop_k > 1:
        din("ktag", [1, cfg.loop_k])             # shape tag to defeat HLO cache
    zp = din("zp", [P, cfg.e_pad // 4], F16)   # local_rel, 4 groups x partitions
    dt = din("dt", [P, cfg.ntiles * H], F16)
    segf = din("segf", [P, cfg.ntiles], F32)
    cntin = din("cntin", [1, cfg.segs_core], F32)
    invb = din("invb", [P, cfg.segs_core], F32)

    outT = nc.dram_tensor("outT", [H, cfg.segs_core], F32,
                          kind="ExternalOutput").ap()
    dbg = {}
    if getattr(cfg, "debug_dump", False):
        for nm, shp in [("d_pre1", [H, 1024]), ("d_h1", [H, 1024]),
                        ("d_pm", [P, 512]), ("d_msgp", [P, 512]),
                        ("d_at", [P, P]), ("d_ST", [H, cfg.segs_core])]:
            dbg[nm] = nc.dram_tensor(nm, shp, F32,
                                     kind="ExternalOutput").ap()

    with tile.TileContext(nc) as tc, ExitStack() as ctx:
        ep = ctx.enter_context

        consts = ep(tc.tile_pool(name="consts", bufs=1))
        zpool = ep(tc.tile_pool(name="zpool", bufs=4))
        gpool = ep(tc.tile_pool(name="gpool", bufs=4))
        hpool = ep(tc.tile_pool(name="hpool", bufs=4))
        mpool = ep(tc.tile_pool(name="mpool", bufs=6))
        apool = ep(tc.tile_pool(name="apool", bufs=12))
        spool = ep(tc.tile_pool(name="spool", bufs=1))
        postp = ep(tc.tile_pool(name="postp", bufs=4))
        # pre1 and pm share one 3-slot rotation (tag "ps", 6 banks): their
        # lifetimes interleave (pre1[k] dies at silu1[k], pm[k] at silu2[k])
        # so 3 slots give both double-buffering within 6 banks, leaving 2
        # banks for the segment accumulators.
        pspool = ep(tc.tile_pool(name="pspool", bufs=3, space="PSUM"))
        psS = ep(tc.tile_pool(name="psS", bufs=2, space="PSUM"))       # seg acc

        def csb(ap_in, shape, dt_=F32, tag=None):
            t = consts.tile(shape, dt_, tag=tag or ap_in.tensor.name)
            nc.sync.dma_start(t[:], ap_in)
            return t

        W1q_sb = csb(W1q, [P, H], F16)
        Wf_sb = csb(Wf, [H, H], F16)
        Ie_sb = csb(Ie, [P, P], F16)
        IOTA_sb = csb(IOTA, [P, P], F16)
        be1_sb = csb(be1c, [H, 1])
        Wm2_sb = csb(Wm2, [H, H])
        bm2_sb = csb(bm2r, [1, H])
        Wu1_sb = csb(Wu1, [H, H])
        bu1_sb = csb(bu1c, [H, 1])
        Wu2_sb = csb(Wu2, [H, H])
        bu2_sb = csb(bu2c, [H, 1])
        segf_sb = consts.tile([P, cfg.ntiles], F32, tag="segf")
        nc.sync.dma_start(segf_sb[:], segf)
        cnt_sb = consts.tile([1, cfg.segs_core], F32, tag="cnt")
        nc.sync.dma_start(cnt_sb[:], cntin)
        invb_sb = consts.tile([P, cfg.segs_core], F32, tag="invb")
        nc.sync.dma_start(invb_sb[:], invb)
        # warm the Silu table set BEFORE the loop so walrus hoists
        # LoadActFuncSet out of the For_i body (else it replays per iter)
        actwarm = consts.tile([H, 1], F32, tag="actwarm")
        nc.scalar.activation(out=actwarm[:], in_=be1_sb[:],
                             func=mybir.ActivationFunctionType.Silu)

        def post_chunk(ST_all, u):
            """mean -> update MLP for segs [u*512, (u+1)*512)."""
            s0 = u * 512
            pmm = pspool.tile([P, 512], F32, tag="ps")
            nc.tensor.matmul(out=pmm[:], lhsT=Wm2_sb[:],
                             rhs=ST_all[:, s0:s0 + 512],
                             start=True, stop=False)
            nc.tensor.matmul(out=pmm[:], lhsT=bm2_sb[:],
                             rhs=cnt_sb[0:1, s0:s0 + 512],
                             start=False, stop=True)
            mean = postp.tile([H, 512], F32, tag="mean")
            nc.vector.tensor_tensor(out=mean[:], in0=pmm[:],
                                    in1=invb_sb[:, s0:s0 + 512],
                                    op=AOT.mult)
            pu = pspool.tile([P, 512], F32, tag="ps")
            nc.tensor.matmul(out=pu[:], lhsT=Wu1_sb[:], rhs=mean[:],
                             start=True, stop=True)
            hu = postp.tile([H, 512], F32, tag="hu")
            nc.scalar.activation(out=hu[:], in_=pu[:],
                                 func=mybir.ActivationFunctionType.Silu,
                                 bias=bu1_sb[:])
            po = pspool.tile([P, 512], F32, tag="ps")
            nc.tensor.matmul(out=po[:], lhsT=Wu2_sb[:], rhs=hu[:],
                             start=True, stop=True)
            ot = postp.tile([H, 512], F32, tag="ot")
            nc.vector.tensor_scalar(out=ot[:], in0=po[:],
                                    scalar1=bu2_sb[:, :1], scalar2=None,
                                    op0=AOT.add)
            nc.sync.dma_start(outT[:, s0:s0 + 512], ot[:])

        loop_cm = tc.For_i(0, cfg.loop_k, 1) if cfg.loop_k > 1 else None
        if loop_cm is not None:
            ctx.enter_context(loop_cm)
        for rep in range(cfg.reps):
            ST_all = spool.tile([H, cfg.segs_core], F32, tag="ST")
            state = {"ps_blk": None}
            nbatch = cfg.nchunks * 2          # 1024-edge batches
            hmap = {}
            dmap = {}
            pending_post = []

            def stage_a(k):
                """DMA (per chunk) + mm1 + silu1 for batch k."""
                c, g = divmod(k, 2)
                if g == 0:
                    zcol = c * CHUNK_TILES * P // 4
                    zpc = zpool.tile([99, CHUNK_TILES * P // 4], F16,
                                     tag="zpc")
                    nc.sync.dma_start(
                        zpc[:], zp[0:99, zcol:zcol + CHUNK_TILES * P // 4])
                    dtc = gpool.tile([P, CHUNK_TILES * H], F16, tag="dtc")
                    nc.sync.dma_start(dtc[:],
                                      dt[:, c * CHUNK_TILES * H:
                                         (c + 1) * CHUNK_TILES * H])
                    dmap[c] = (zpc, dtc)
                zpc, dtc = dmap[c]
                pre1 = pspool.tile([H, 1024], F32, tag="ps")
                # two row-tiled K=3 matmuls run concurrently on the PE
                for jj in range(2):
                    r0 = 64 * g + 32 * jj
                    nc.tensor.matmul(out=pre1[:, jj * 512:(jj + 1) * 512],
                                     lhsT=W1q_sb[r0:r0 + 3, :],
                                     rhs=zpc[r0:r0 + 3, :],
                                     start=True, stop=True,
                                     tile_position=(r0, 0))
                h1 = hpool.tile([H, 1024], F16, tag="h1")
                nc.scalar.activation(out=h1[:], in_=pre1[:],
                                     func=mybir.ActivationFunctionType.Silu,
                                     bias=be1_sb[:])
                hmap[k] = h1

            def stage_b(k):
                """mm2 + silu2 + segment reduction for batch k."""
                c, g = divmod(k, 2)
                _, dtc = dmap[c]
                h1 = hmap.pop(k)
                # one pm tile spanning 2 banks; per bank: D matmul first
                # with start=True (start clears has_written bank-wide),
                # then the Wf matmuls accumulate per-quarter.
                pm = pspool.tile([P, 1024], F32, tag="ps")
                for half in range(2):
                    hofs = half * 512
                    dts = dtc[:, (g * 8 + half * 4) * H:
                              (g * 8 + half * 4 + 4) * H]
                    nc.tensor.matmul(out=pm[:, hofs:hofs + 512],
                                     lhsT=Ie_sb[:], rhs=dts,
                                     start=True, stop=False,
                                     skip_group_check=True)
                    for t4 in range(4):
                        co = hofs + t4 * 128
                        nc.tensor.matmul(
                            out=pm[:, co:co + 128],
                            lhsT=h1[:, co:co + 128],
                            rhs=Wf_sb[:], start=False, stop=True,
                            skip_group_check=True)
                msgp = mpool.tile([P, 1024], F16, tag="msgp")
                nc.scalar.activation(
                    out=msgp[:], in_=pm[:],
                    func=mybir.ActivationFunctionType.Silu)
                t0 = c * CHUNK_TILES + g * 8
                for t8 in range(8):
                    t = t0 + t8
                    s = cfg.tile_slot[t]
                    at = apool.tile([P, P], F16, tag="at")
                    nc.vector.tensor_scalar(
                        out=at[:], in0=IOTA_sb[:],
                        scalar1=segf_sb[:, t:t + 1], scalar2=None,
                        op0=AOT.is_equal)
                    if cfg.tile_first[t]:
                        ps_new = psS.tile([H, P], F32, tag="psS")
                        state["ps_blk"] = ps_new
                    nc.tensor.matmul(
                        out=state["ps_blk"][:],
                        lhsT=msgp[:, t8 * 128:t8 * 128 + 128],
                        rhs=at[:],
                        start=cfg.tile_first[t],
                        stop=cfg.tile_last[t])
                    if cfg.tile_last[t]:
                        nc.vector.tensor_copy(
                            out=ST_all[:, s * BLK:(s + 1) * BLK],
                            in_=state["ps_blk"][:])
                        # queue post for a 512-seg range once its 4 slots
                        # are reduced; emitted one batch later so the
                        # PE/DVE chain completes before hu hits the Act FIFO
                        if (s + 1) % 4 == 0:
                            pending_post.append(s // 4)

            # software pipeline: stage_a one batch ahead of stage_b;
            # queued post chunks flush after the NEXT batch's stage_b
            post_mode = getattr(cfg, "post_mode", "defer")
            for k in range(nbatch + 1):
                if k < nbatch:
                    stage_a(k)
                if post_mode == "defer":
                    flush = list(pending_post)
                    pending_post.clear()
                if k >= 1:
                    stage_b(k - 1)
                if post_mode == "defer":
                    for u in flush:
                        post_chunk(ST_all, u)
            for u in pending_post:
                post_chunk(ST_all, u)
            pending_post.clear()

            if dbg and rep == 0:
                nc.sync.dma_start(dbg["d_ST"], ST_all[:])

    nc.compile()
    return nc


# ======================= host preprocessing =======================

def silu_np(x):
    return x / (1.0 + np.exp(-x))


def host_prep(inputs, ncores, use_bf16=False, t_override=None):
    """Returns (cfg, list of per-core in_maps, const row for node outputs)."""
    nemb = np.asarray(inputs["node_embedding"], np.float32)
    npos = np.asarray(inputs["node_pos"], np.float32)
    gpos = np.asarray(inputs["grid_pos"], np.float32)
    eidx = np.asarray(inputs["edge_index"], np.int64)
    frames = np.asarray(inputs["equi_frames"], np.float32)
    batch = np.asarray(inputs["batch"], np.int64)
    We1 = np.asarray(inputs["We1"], np.float32); be1 = np.asarray(inputs["be1"], np.float32)
    We2 = np.asarray(inputs["We2"], np.float32); be2 = np.asarray(inputs["be2"], np.float32)
    Wm1 = np.asarray(inputs["Wm1"], np.float32); bm1 = np.asarray(inputs["bm1"], np.float32)
    Wm2 = np.asarray(inputs["Wm2"], np.float32); bm2 = np.asarray(inputs["bm2"], np.float32)
    Wu1 = np.asarray(inputs["Wu1"], np.float32); bu1 = np.asarray(inputs["bu1"], np.float32)
    Wu2 = np.asarray(inputs["Wu2"], np.float32); bu2 = np.asarray(inputs["bu2"], np.float32)

    N, Hh = nemb.shape
    G = gpos.shape[0]
    E = eidx.shape[1]
    B = frames.shape[0]
    assert Hh == H

    i_all = eidx[0]
    jg_all = eidx[1] - N
    order = np.argsort(jg_all, kind="stable")
    jg_s = jg_all[order]
    i_s = i_all[order]

    segs_core = G // ncores
    nslots = segs_core // BLK
    nblk_g = G // BLK
    gb = jg_s // BLK                                  # global block per edge
    counts_blk = np.bincount(gb, minlength=nblk_g)
    per_core_cnt = counts_blk.reshape(ncores, nslots)

    # per-core block permutation: descending count; shared static schedule
    perms = [np.argsort(per_core_cnt[c], kind="stable")[::-1]
             for c in range(ncores)]
    sorted_cnt = np.stack([per_core_cnt[c][perms[c]] for c in range(ncores)])
    slot_tiles = np.ceil(sorted_cnt.max(axis=0) / P).astype(int)
    slot_tiles = np.maximum(slot_tiles, 1)
    ntiles = int(slot_tiles.sum())
    pad = (-ntiles) % CHUNK_TILES
    slot_tiles[-1] += pad
    cfg = Cfg(N, G, E, B, ncores, slot_tiles.tolist())
    slot_off = np.zeros(nslots + 1, np.int64)
    slot_off[1:] = np.cumsum(slot_tiles)

    # per-edge host data (pure data movement + O(N)/O(B)/O(E) elementwise math)
    R_flat = frames.reshape(B, 9)
    b_e = batch[i_s]
    gp_e = gpos[jg_s]                                  # [E, 3]
    zr = R_flat[b_e].reshape(E, 3, 3)                  # [E, 3, 3]
    rp_node = np.einsum("nab,nb->na", frames[batch], npos).astype(np.float32)
    lr_e = (zr * gp_e[:, None, :]).sum(-1) - rp_node[i_s]   # local_rel [E, 3]

    bmix = bm1 + be2 @ Wm1[H:]
    Dtab = (nemb @ Wm1[:H] + bmix[None, :]).astype(np.float32)   # [N, H]
    D_e = Dtab[i_s]                                    # [E, H]

    ecount = np.bincount(jg_all, minlength=G).astype(np.float32)

    W1q = np.zeros((P, H), np.float16)
    for J in range(4):
        W1q[32 * J:32 * J + 3] = We1.astype(np.float16)
    shared = {
        "W1q": W1q,
        "Wf": np.ascontiguousarray(We2 @ Wm1[H:]).astype(np.float16),
        "Ie": np.eye(P, dtype=np.float16),
        "IOTA": np.ascontiguousarray(
            np.tile(np.arange(P, dtype=np.float16)[None, :], (P, 1))),
        "be1c": np.ascontiguousarray(be1[:, None]),
        "Wm2": np.ascontiguousarray(Wm2),
        "bm2r": np.ascontiguousarray(bm2[None, :]),
        "Wu1": np.ascontiguousarray(Wu1),
        "bu1c": np.ascontiguousarray(bu1[:, None]),
        "Wu2": np.ascontiguousarray(Wu2),
        "bu2c": np.ascontiguousarray(bu2[:, None]),
    }

    core_of_edge = gb // nslots
    in_maps = []
    for c in range(ncores):
        sel = core_of_edge == c
        jg_c = jg_s[sel]
        b_local = (gb[sel] % nslots)                   # original block id
        # slot of each edge + position within slot
        inv_perm = np.empty(nslots, np.int64)
        inv_perm[perms[c]] = np.arange(nslots)
        slot_e = inv_perm[b_local]
        # rank within block: edges sorted by jg so within-block order stable
        cnts = per_core_cnt[c][perms[c]]
        # compute start offset of each edge within its block
        blk_start = np.zeros(nblk_g + 1, np.int64)
        blk_start[1:] = np.cumsum(counts_blk)
        rank = np.nonzero(sel)[0] - blk_start[gb[sel]]
        slot_pos = slot_off[slot_e] * P + rank

        e_pad = cfg.e_pad
        lr_c = np.zeros((e_pad, 3), np.float32)
        lr_c[slot_pos] = lr_e[sel]
        dt_c = np.zeros((e_pad, H), np.float32)
        dt_c[slot_pos] = D_e[sel]
        segf_f = np.full(e_pad, -1.0, np.float32)
        segf_f[slot_pos] = (jg_c % BLK).astype(np.float32)

        # device layouts: zp_t[32J + a, c*512 + col] = lr_a of edge
        # c*2048 + J*512 + col (partition-packed quarters of each chunk)
        v = lr_c.reshape(cfg.nchunks, 4, 512, 3)
        zp_t = np.zeros((P, e_pad // 4), np.float16)
        for J in range(4):
            for a in range(3):
                zp_t[32 * J + a] = v[:, J, :, a].reshape(-1)
        dt_t = np.ascontiguousarray(
            dt_c.reshape(cfg.ntiles, P, H).transpose(1, 0, 2)
            .reshape(P, cfg.ntiles * H)).astype(np.float16)
        segf_t = np.ascontiguousarray(
            segf_f.reshape(cfg.ntiles, P).T)                     # [128, ntiles]

        cnt_core = ecount[c * segs_core:(c + 1) * segs_core]
        cnt_slot = cnt_core.reshape(nslots, BLK)[perms[c]].reshape(-1)
        inv_slot = 1.0 / np.maximum(cnt_slot, 1.0)

        m = dict(shared)
        m["zp"] = zp_t
        m["dt"] = dt_t
        m["segf"] = segf_t
        m["cntin"] = np.ascontiguousarray(cnt_slot[None, :].astype(np.float32))
        m["invb"] = np.ascontiguousarray(
            np.tile(inv_slot[None, :], (P, 1)).astype(np.float32))
        in_maps.append(m)

    const_row = silu_np(bu1) @ Wu2 + bu2
    cfg.perms = perms
    return cfg, in_maps, const_row


def assemble_output(cfg, results, const_row, N, G):
    out = np.empty((N + G, H), np.float32)
    out[:N] = const_row[None, :]
    nslots = cfg.nslots
    for c in range(cfg.ncores):
        res = results[c]["outT"].T                    # [segs_core, H] slot order
        dest = out[N + c * cfg.segs_core: N + (c + 1) * cfg.segs_core]
        dest.reshape(nslots, BLK, H)[cfg.perms[c]] = res.reshape(nslots, BLK, H)
    return out


# ======================= top-level kernel entry =======================

_PROGRAM_CACHE = {}

NCORES = 8
USE_BF16 = False


def kernel(**inputs):
    """Full-input entry point: shards edges by destination grid cell across
    8 NeuronCores, runs the Bass/Tile program, reassembles the full output."""
    from concourse.bass_utils import run_bass_kernel_spmd

    cfg, in_maps, const_row = host_prep(inputs, NCORES, use_bf16=USE_BF16)
    key = cfg.key()
    if key not in _PROGRAM_CACHE:
        _PROGRAM_CACHE[key] = build_program(cfg)
    nc = _PROGRAM_CACHE[key]
    res = run_bass_kernel_spmd(nc, in_maps, core_ids=list(range(NCORES)))
    N = inputs["node_pos"].shape[0]
    G = inputs["grid_pos"].shape[0]
    return assemble_output(cfg, res.results, const_row, N, G)
